# revision 5
# baseline (speedup 1.0000x reference)
"""BiAttention (binary attention transformer block) Trainium2 kernel.

Forward-pass reduction of the reference:
  - softmax cancels:  stop_gradient(binq - soft) + soft == binq  (forward)
  - sign() is invariant to the positive per-row qkv weight scale
So per batch element (one per NeuronCore, 8 cores data-parallel):
  bq,bk,bv = sign(x @ sign(Wqkv).T)   split into heads
  A        = (bq @ bk.T > 0)          in {0,1}
  oo       = A @ bv                   exact small integers
  out      = (oo @ sign(Wproj).T) * mean(|Wproj|,axis=1) + b_proj

Host-side prep (layout only, no matmul FLOPs): x is transposed and split
into fp16 hi/lo (two-pass fp16 matmul == exact fp32 qkv signs), weight
signs are precomputed as fp16 (+-1), the proj scale is folded into the
proj weight signs, and all tensors are passed pre-transposed so the
device spends zero PE cycles on transposes.

Device layout notes:
  - qkv computed transposed [o, n] directly (lhsT=wsT chunk, rhs=xT)
  - v-part computed natural [m, hd] (lhsT=xT chunk, rhs=wsT v columns)
  - scores computed per head as maskT [m, n] fp8 {0,1}
  - A@V via fp8 DoubleRow, odd head lands on PSUM partitions 64-127 so
    a single [128,512] evac covers a head pair
"""

import numpy as np

import concourse.bacc as bacc
import concourse.mybir as mybir
import concourse.tile as tile

FP32 = mybir.dt.float32
FP16 = mybir.dt.float16
FP8 = mybir.dt.float8e4
AF = mybir.ActivationFunctionType
ALU = mybir.AluOpType
DR = mybir.MatmulPerfMode.DoubleRow

B, N, C = 8, 1024, 768
H, D = 12, 64
C3 = 3 * C  # 2304
NK = C // 128  # 6 contraction chunks
NM = N // 128  # 8 token chunks


def build_nc(repeat=1):
    nc = bacc.Bacc("TRN2", target_bir_lowering=False, debug=True)

    # host-prepped inputs (see kernel() below)
    xhi_d = nc.dram_tensor("xt_hi", [C, N], FP16, kind="ExternalInput")
    xlo_d = nc.dram_tensor("xt_lo", [C, N], FP16, kind="ExternalInput")
    # wsT columns reordered: [ v (768) | hp0: q(128) k(128) | hp1: ... ]
    wst_d = nc.dram_tensor("wst", [C, C3], FP16, kind="ExternalInput")
    w2t_d = nc.dram_tensor("w2t", [C, C], FP16, kind="ExternalInput")  # scaled
    bias_d = nc.dram_tensor("bias", [1, C], FP32, kind="ExternalInput")
    out_d = nc.dram_tensor("out", [N, C], FP32, kind="ExternalOutput")

    xhi_v = xhi_d[:].rearrange("(c p) n -> p c n", p=128)  # [128, 6, 1024]
    xlo_v = xlo_d[:].rearrange("(c p) n -> p c n", p=128)
    wst_v = wst_d[:].rearrange("(c p) f -> p c f", p=128)  # [128, 6, 2304]
    w2t_v = w2t_d[:].rearrange("(c p) f -> p c f", p=128)  # [128, 6, 768]
    out_v = out_d[:].rearrange("(m p) f -> p m f", p=128)  # [128, 8, 768]

    with tile.TileContext(nc) as tc:
        for _rep in range(repeat):
            _emit_body(nc, tc, _rep, xhi_v, xlo_v, wst_v, w2t_v, bias_d, out_v)

    nc.compile()
    return nc


def _emit_body(nc, tc, rep, xhi_v, xlo_v, wst_v, w2t_v, bias_d, out_v):
    _p = f"r{rep}_"
    with (
        tc.tile_pool(name=_p + "persist", bufs=1) as pp,
        tc.tile_pool(name=_p + "qk", bufs=5) as qkp,
        tc.tile_pool(name=_p + "at", bufs=4) as atp,
        tc.tile_pool(name=_p + "outstage", bufs=3) as op,
    ):
        # ---- persistent SBUF ----
        xT_hi = pp.tile([128, NK, N], FP16, tag="xT_hi")  # [c%128, c//128, n]
        xT_lo = pp.tile([128, NK, N], FP16, tag="xT_lo")
        wsT = pp.tile([128, NK, C3], FP16, tag="wsT")
        w2T = pp.tile([128, NK, C], FP16, tag="w2T")
        v_nat = pp.tile([128, NM, C], FP8, tag="v_nat")  # v, ±0.5, [m%128, m//128, hd]
        ooT = pp.tile([128, NK, N], FP16, tag="ooT")  # attn out transposed
        bias_row = pp.tile([1, C], FP32, tag="bias_row")
        bias_rep = pp.tile([128, C], FP32, tag="bias_rep")
        sigb = pp.tile([128, 1], FP32, tag="sigb")

        nc.gpsimd.memset(sigb[:], -32.0)
        nc.sync.dma_start(bias_row[:], bias_d[:])
        nc.gpsimd.partition_broadcast(bias_rep[:], bias_row[:])

        # ---- input DMAs (ordered so compute can start early) ----
        for cc in range(NK):
            nc.sync.dma_start(xT_hi[:, cc, :], xhi_v[:, cc, :])
        for cc in range(NK):
            nc.sync.dma_start(xT_lo[:, cc, :], xlo_v[:, cc, :])
        # v columns first (v-part of qkv runs first), then per-head-pair slabs
        nc.sync.dma_start(wsT[:, :, 0:768], wst_v[:, :, 0:768])
        for hp in range(6):
            o0 = 768 + hp * 256
            nc.sync.dma_start(wsT[:, :, o0 : o0 + 256], wst_v[:, :, o0 : o0 + 256])
        nc.sync.dma_start(w2T[:], w2t_v[:])

        qkv_srcs = (xT_hi, xT_lo)
        ns = len(qkv_srcs)

        # ---- v-part: natural orientation [m, hd], fp8 ±0.5 ----
        vp_cm = tc.tile_pool(name=_p + "ps_v", bufs=2, space="PSUM")
        ps_v = vp_cm.__enter__()
        for m in range(NM):
            for half in range(2):
                vp = ps_v.tile([128, 384], FP32, tag="v_ps", name=f"vps{m}_{half}")
                for k in range(NK):
                    for si, src in enumerate(qkv_srcs):
                        nc.tensor.matmul(
                            vp[:],
                            lhsT=src[:, k, m * 128 : (m + 1) * 128],
                            rhs=wsT[:, k, half * 384 : (half + 1) * 384],
                            start=(k == 0 and si == 0),
                            stop=(k == NK - 1 and si == ns - 1),
                        )
                nc.vector.tensor_scalar(
                    v_nat[:, m, half * 384 : (half + 1) * 384],
                    vp[:],
                    0.0,
                    0.5,
                    ALU.is_ge,
                    ALU.subtract,
                )
        vp_cm.__exit__(None, None, None)

        # ---- per head-quad: q/k qkv chunks, scores (fp8 DR, 4 row-groups),
        # binarize, A@V (2 col-groups) ----
        hp_psum_cms = [
            tc.tile_pool(name=_p + "ps_qk", bufs=2, space="PSUM"),
            tc.tile_pool(name=_p + "ps_s", bufs=4, space="PSUM"),
            tc.tile_pool(name=_p + "ps_oo", bufs=2, space="PSUM"),
        ]
        ps_qk, ps_s, ps_oo = [cm.__enter__() for cm in hp_psum_cms]
        bin_idx = 0
        qkTs = {}

        def emit_qk(g):
            # per quad g: q/k signs in DoubleRow layout [128, 2, N] fp8:
            # partition 32*hi + (d%32), pair index j = d//32 (hi = head in quad)
            qkT = {}
            for ri, role in enumerate(("q", "k")):
                t = qkp.tile([128, 2, N], FP8, tag="qkT", name=f"qkT_{role}{g}")
                qkT[role] = t
                for j in range(2):
                    oc0 = 768 + g * 1024 + ri * 512 + j * 128
                    for ncol in range(2):
                        qp = ps_qk.tile([128, 512], FP32, tag="qk_ps")
                        for k in range(NK):
                            for si, src in enumerate(qkv_srcs):
                                nc.tensor.matmul(
                                    qp[:],
                                    lhsT=wsT[:, k, oc0 : oc0 + 128],
                                    rhs=src[:, k, ncol * 512 : (ncol + 1) * 512],
                                    start=(k == 0 and si == 0),
                                    stop=(k == NK - 1 and si == ns - 1),
                                )
                        nc.scalar.activation(
                            t[:, j, ncol * 512 : (ncol + 1) * 512], qp[:], AF.Sign
                        )
            qkTs[g] = qkT

        emit_qk(0)
        for g in range(3):
            qkT = qkTs.pop(g)
            at = [
                atp.tile([128, NM, N], FP8, tag="at", name=f"at{g}_{hi}")
                for hi in range(4)
            ]
            sps = [
                ps_s.tile([128, 512], FP32, tag="s_ps", name=f"sps{g}_{hi}")
                for hi in range(4)
            ]
            for m in range(NM):
                for ncol in range(2):
                    for hi in range(4):
                        nc.tensor.matmul(
                            sps[hi][:],
                            lhsT=qkT["k"][32 * hi : 32 * hi + 32, :, m * 128 : (m + 1) * 128],
                            rhs=qkT["q"][32 * hi : 32 * hi + 32, :, ncol * 512 : (ncol + 1) * 512],
                            perf_mode=DR,
                            tile_position=(32 * hi, 0),
                        )
                    for hi in range(4):
                        dst = at[hi][:, m, ncol * 512 : (ncol + 1) * 512]
                        if bin_idx % 2 == 0:
                            nc.scalar.activation(
                                dst, sps[hi][:], AF.Sigmoid, bias=sigb[:], scale=32.0
                            )
                        else:
                            nc.vector.tensor_scalar(dst, sps[hi][:], 0.0, None, ALU.is_gt)
                        bin_idx += 1
                        if m < NM - 1 or ncol < 1:
                            sps[hi] = ps_s.tile(
                                [128, 512], FP32, tag="s_ps", name=f"sps{g}_{m}_{ncol}_{hi}"
                            )

            if g + 1 < 3:
                emit_qk(g + 1)

            # A@V: even head -> PSUM rows 0-63 (col group 0), odd head ->
            # rows 64-127 (col group 64); the two chains run concurrently and
            # a single [128,512] evac covers the head pair.
            for hp_in in range(2):
                hp = 2 * g + hp_in
                he = 4 * g + 2 * hp_in
                for ncol in range(2):
                    oo_ps = ps_oo.tile(
                        [128, 512], FP32, tag="oo_ps", name=f"oo_ps{hp}_{ncol}"
                    )
                    for j in range(NM):
                        for h01 in range(2):
                            h = he + h01
                            nc.tensor.matmul(
                                oo_ps[h01 * 64 : h01 * 64 + 64, :],
                                lhsT=v_nat[:, j, h * 64 : (h + 1) * 64],
                                rhs=at[2 * hp_in + h01][:, j, ncol * 512 : (ncol + 1) * 512],
                                start=(j == 0),
                                stop=(j == NM - 1),
                            )
                    # v was ±0.5 -> x2 recovers exact integer attention out
                    csl = ooT[:, hp, ncol * 512 : (ncol + 1) * 512]
                    if ncol == 0:
                        nc.scalar.activation(csl, oo_ps[:], AF.Copy, scale=2.0)
                    else:
                        nc.vector.tensor_scalar(csl, oo_ps[:], 2.0, None, ALU.mult)
        for cm in reversed(hp_psum_cms):
            cm.__exit__(None, None, None)

        # ---- projection (scale pre-folded into w2T on host) ----
        with tc.tile_pool(name=_p + "ps_proj", bufs=2, space="PSUM") as ps_p:
            for m in range(NM):
                ot = op.tile([128, C], FP32, tag="out_stage")
                for n0, nw in ((0, 512), (512, 256)):
                    pps = ps_p.tile([128, nw], FP32, tag=f"p_ps{n0}")
                    for k in range(NK):
                        nc.tensor.matmul(
                            pps[:],
                            lhsT=ooT[:, k, m * 128 : (m + 1) * 128],
                            rhs=w2T[:, k, n0 : n0 + nw],
                            start=(k == 0),
                            stop=(k == NK - 1),
                        )
                    nc.vector.scalar_tensor_tensor(
                        ot[:, n0 : n0 + nw],
                        pps[:],
                        1.0,
                        bias_rep[:, n0 : n0 + nw],
                        ALU.bypass,
                        ALU.add,
                    )
                nc.sync.dma_start(out_v[:, m, :], ot[:])


_CACHE = {}


def _get_exec():
    """Build (once) and cache a jitted SPMD executable for the 8-core kernel."""
    if "exec" in _CACHE:
        return _CACHE["exec"]
    import jax
    import concourse.mybir as _mybir
    from jax.sharding import Mesh, PartitionSpec
    from jax.experimental.shard_map import shard_map
    from concourse.bass2jax import _bass_exec_p, install_neuronx_cc_hook

    nc = build_nc()
    install_neuronx_cc_hook()

    in_names, out_names, out_avals = [], [], []
    for alloc in nc.m.functions[0].allocations:
        if not isinstance(alloc, _mybir.MemoryLocationSet):
            continue
        name = alloc.memorylocations[0].name
        if alloc.kind == "ExternalInput":
            if name not in ("dbg_addr", "partition_id"):
                in_names.append(name)
        elif alloc.kind == "ExternalOutput":
            out_names.append(name)
            out_avals.append(
                jax.core.ShapedArray(tuple(alloc.tensor_shape), _mybir.dt.np(alloc.dtype))
            )
    if nc.dbg_addr is not None:
        in_names.append(nc.dbg_addr.name)
    n_params = len(in_names)
    n_outs = len(out_names)
    partition_name = nc.partition_id_tensor.name if nc.partition_id_tensor else None
    all_in_names = tuple(
        in_names + out_names + ([partition_name] if partition_name else [])
    )
    donate = tuple(range(n_params, n_params + n_outs))

    def _body(*args):
        operands = list(args)
        if partition_name is not None:
            from concourse.bass2jax import partition_id_tensor

            operands.append(partition_id_tensor())
        outs = _bass_exec_p.bind(
            *operands,
            out_avals=tuple(out_avals),
            in_names=all_in_names,
            out_names=tuple(out_names),
            lowering_input_output_aliases=(),
            sim_require_finite=True,
            sim_require_nnan=True,
            nc=nc,
        )
        return tuple(outs)

    devices = jax.devices()[:B]
    mesh = Mesh(np.array(devices), ("core",))
    in_specs = (PartitionSpec("core"),) * (n_params + n_outs)
    out_specs = (PartitionSpec("core"),) * n_outs
    sharded = jax.jit(
        shard_map(_body, mesh=mesh, in_specs=in_specs, out_specs=out_specs, check_rep=False),
        donate_argnums=donate,
        keep_unused=True,
    )
    _CACHE["exec"] = (sharded, in_names, out_names, out_avals, mesh)
    return _CACHE["exec"]


def _host_prep(x, w_qkv, w_proj, b_proj):
    """Layout-only host prep: transposes, fp16 hi/lo split, weight signs."""
    x = np.asarray(x, np.float32)
    w_qkv = np.asarray(w_qkv, np.float32)
    w_proj = np.asarray(w_proj, np.float32)
    b_proj = np.asarray(b_proj, np.float32).reshape(1, C)

    # qkv weight signs, transposed, with columns reordered:
    # [ v (768) | hp0: q(128) k(128) | hp1: q k | ... ]
    ws = np.where(w_qkv >= 0, np.float16(1.0), np.float16(-1.0))  # [2304, 768]
    q_s, k_s, v_s = ws[0:C], ws[C : 2 * C], ws[2 * C :]
    cols = [v_s]
    for hp in range(6):
        cols.append(q_s[hp * 128 : (hp + 1) * 128])
        cols.append(k_s[hp * 128 : (hp + 1) * 128])
    wst = np.ascontiguousarray(np.concatenate(cols, axis=0).T)  # [768, 2304] fp16

    # proj: fold per-row scale into the sign matrix (fp16 rounding of the
    # scale is ~2^-12 relative -- far inside tolerance)
    sc2 = np.abs(w_proj).mean(axis=1, dtype=np.float64).astype(np.float32)
    w2 = np.where(w_proj >= 0, 1.0, -1.0).astype(np.float32) * sc2[:, None]
    w2t = np.ascontiguousarray(w2.T.astype(np.float16))  # [768, 768]

    # x per batch: transpose, fp16 hi/lo split
    xt = np.ascontiguousarray(x.transpose(0, 2, 1))  # [B, 768, 1024]
    xt_hi = xt.astype(np.float16)
    xt_lo = (xt - xt_hi.astype(np.float32)).astype(np.float16)
    return xt_hi, xt_lo, wst, w2t, b_proj


def _concat_inputs(x, w_qkv, w_proj, b_proj):
    """Per-core inputs concatenated along axis 0 (shard_map convention)."""
    xt_hi, xt_lo, wst, w2t, bias = _host_prep(x, w_qkv, w_proj, b_proj)
    per_core = {
        "xt_hi": [np.ascontiguousarray(xt_hi[b]) for b in range(B)],
        "xt_lo": [np.ascontiguousarray(xt_lo[b]) for b in range(B)],
        "wst": [wst] * B,
        "w2t": [w2t] * B,
        "bias": [bias] * B,
        "dbg_addr": [np.zeros((1, 2), np.uint32)] * B,
    }
    return per_core


def _zero_outs(out_names, out_avals):
    return [
        np.zeros((B * a.shape[0], *a.shape[1:]), a.dtype) for a in out_avals
    ]


def kernel(x, w_qkv, w_proj, b_proj):
    sharded, in_names, out_names, out_avals, mesh = _get_exec()
    per_core = _concat_inputs(x, w_qkv, w_proj, b_proj)
    concat_in = [np.concatenate(per_core[name], axis=0) for name in in_names]
    out_arrs = sharded(*concat_in, *_zero_outs(out_names, out_avals))
    i = out_names.index("out")
    a = out_avals[i]
    return np.asarray(out_arrs[i]).reshape(B, *a.shape)
